# revision 1
# baseline (speedup 1.0000x reference)
"""LogSparseAttention Trainium2 kernel.

B,L,H,E = 2,2048,8,64 ; S,D = 2048,64 ; fp32 in/out.
Shard B*H = 16 (b,h) pairs across 8 cores, 2 pairs/core.

Per (b,h): scores^T[j,i] = K[j]·Q[i] computed only on a sparse set of
"stripe" windows per 128-row j-chunk:
  band  : i in [128c, 128c+272)          covers delta = i-j in {0..12,14,18,26,42,74,138} (+rows i<22 full causal)
  far-d : i in [128c+d, 128c+d+128)      covers delta == d for d in {266, 522, 1034}
exp on ScalarE (no max-subtraction needed: |score*scale| <= ~6), 0/1 bf16
mask multiply on VectorE, then PV matmuls (V augmented with a ones column
so row 64 of O^T accumulates Z) accumulate into a PSUM O^T [65, 2048].
Epilogue: copy->SBUF, PE-transpose per 128-col tile, multiply by 1/Z,
DMA out. Matmuls in bf16 (inputs pre-cast on host).
"""

import math

import ml_dtypes
import numpy as np

B, L, H, E = 2, 2048, 8, 64
S, D = 2048, 64
NC_CORES = 8
PAIRS_PER_CORE = 2
CH = L // 128  # 16 chunks
SCALE = 1.0 / math.sqrt(E)

WB = 272                      # band window width
FARS = (266, 522, 1034)       # far diagonals, 128-wide windows each
WIN_A = WB + 128              # psA: band | far266
WIN_B = 256                   # psB: far522 | far1034
WTOT = WIN_A + WIN_B          # 656 mask row width per chunk
QTW = 3104                    # QT padded width >= 128*15 + 1034 + 128 = 3082

BAND_SET = frozenset(list(range(0, 13)) + [14, 18, 26, 42, 74, 138])


# ---------------------------------------------------------------- host masks
def _full_mask() -> np.ndarray:
    """Replica of the reference log-sparse mask [L, S] (0/1 float32)."""
    log_l = math.ceil(math.log2(L))
    m = np.zeros((L, S), dtype=np.float32)
    for index in range(L):
        row = np.zeros(S, dtype=np.float32)
        if (S // L) * 2 * log_l > index:
            row[: index + 1] = 1.0
        else:
            idx = index
            while idx >= 0:
                if idx - log_l + 1 < 0:
                    row[:idx] = 1.0
                    break
                row[idx - log_l + 1 : idx + 1] = 1.0
                for i in range(log_l):
                    new_index = idx - log_l + 1 - 2**i
                    if idx - new_index <= L and new_index >= 0:
                        row[new_index] = 1.0
                idx -= L
        m[index] = row
    return m


def _window_masks():
    """Per-chunk [128, WTOT] 0/1 masks in S^T orientation, deduplicated.

    Returns (masks_np [128, ndist*WTOT] bf16, idx_per_chunk list[int]).
    Also asserts the windows exactly tile the reference mask.
    """
    mf = _full_mask()
    scatter = np.zeros_like(mf)
    per_c = []
    for c in range(CH):
        m = np.zeros((128, WTOT), dtype=np.float32)
        j0 = 128 * c
        # band: cols [0, WB): i = j0 + f, j = j0 + p
        for p in range(128):
            j = j0 + p
            for f in range(WB):
                i = j0 + f
                if i >= L or j > i:
                    continue
                d = i - j
                if (i < 22 and j <= i) or (i >= 22 and d in BAND_SET):
                    if mf[i, j] != 1.0:
                        raise AssertionError(f"band mask mismatch i={i} j={j}")
                    m[p, f] = 1.0
                    scatter[i, j] += 1.0
        # far windows
        for wi, dd in enumerate(FARS):
            off = WB + 128 * wi
            for p in range(128):
                j = j0 + p
                i = j + dd
                if i >= L:
                    continue
                if mf[i, j] != 1.0:
                    raise AssertionError(f"far mask mismatch i={i} j={j}")
                m[p, off + p] = 1.0
                scatter[i, j] += 1.0
        per_c.append(m)
    if not np.array_equal(scatter, mf):
        bad = np.argwhere(scatter != mf)
        raise AssertionError(f"window masks do not tile reference mask: {bad[:5]}")
    # dedupe
    distinct, idx_per_chunk = [], []
    seen = {}
    for m in per_c:
        key = m.tobytes()
        if key not in seen:
            seen[key] = len(distinct)
            distinct.append(m)
        idx_per_chunk.append(seen[key])
    masks_np = np.concatenate(distinct, axis=1).astype(ml_dtypes.bfloat16)
    return masks_np, idx_per_chunk


_MASKS_NP, _MASK_IDX = _window_masks()
_NDIST = _MASKS_NP.shape[1] // WTOT


# ---------------------------------------------------------------- PV pieces
def _pv_pieces(c):
    """PV matmul pieces for chunk c.

    Each: (tile_id 'A'|'B', src_off, dst_start, width, stop).
    dst ranges clipped to [0, L), split at 512-col PSUM bank bounds; the
    band's first 128 cols form their own piece with stop=True (last writer
    of O^T cols [128c, 128c+128)).
    """
    pieces = []

    def add(tile_id, src_off, dst_start, width, stop):
        if dst_start >= L:
            return
        width = min(width, L - dst_start)
        if width <= 0:
            return
        a = dst_start
        while a < dst_start + width:
            b = min(dst_start + width, (a // 512 + 1) * 512)
            pieces.append((tile_id, src_off + (a - dst_start), a, b - a, stop))
            a = b

    j0 = 128 * c
    add("A", 0, j0, 128, True)            # band head (stop)
    add("A", 128, j0 + 128, WB - 128, False)  # band tail
    add("A", WB, j0 + FARS[0], 128, False)
    add("B", 0, j0 + FARS[1], 128, False)
    add("B", 128, j0 + FARS[2], 128, False)
    return pieces


# ---------------------------------------------------------------- bass build
_CACHE = {}


def _build_nc():
    import concourse.bacc as bacc
    import concourse.bass as bass
    import concourse.mybir as mybir
    import concourse.tile as tile

    f32 = mybir.dt.float32
    bf16 = mybir.dt.bfloat16
    AF = mybir.ActivationFunctionType

    nc = bacc.Bacc()
    q_d = nc.dram_tensor("q", [PAIRS_PER_CORE, L, E], bf16, kind="ExternalInput")
    k_d = nc.dram_tensor("k", [PAIRS_PER_CORE, S, E], bf16, kind="ExternalInput")
    v_d = nc.dram_tensor("v", [PAIRS_PER_CORE, S, D], bf16, kind="ExternalInput")
    m_d = nc.dram_tensor("masks", [128, _NDIST * WTOT], bf16, kind="ExternalInput")
    i_d = nc.dram_tensor("ident", [65, 65], f32, kind="ExternalInput")
    o_d = nc.dram_tensor("out", [PAIRS_PER_CORE, L, D], f32, kind="ExternalOutput")

    with tile.TileContext(nc) as tc:
        with (
            tc.tile_pool(name="const", bufs=1) as constp,
            tc.tile_pool(name="io", bufs=2) as iop,
            tc.tile_pool(name="sc", bufs=3) as scp,
            tc.tile_pool(name="ps", bufs=2, space=bass.MemorySpace.PSUM) as psp,
            tc.tile_pool(name="ot", bufs=1, space=bass.MemorySpace.PSUM) as otp,
        ):
            masks = constp.tile([128, _NDIST * WTOT], bf16)
            nc.sync.dma_start(masks[:], m_d[:])
            ident = constp.tile([65, 65], f32)
            nc.sync.dma_start(ident[:], i_d[:])
            zc = constp.tile([1, 65], bf16)
            nc.vector.memset(zc[:], 0.0)
            zr = constp.tile([1, 512], bf16)
            nc.vector.memset(zr[:], 0.0)

            for hh in range(PAIRS_PER_CORE):
                qt = iop.tile([64, QTW], bf16, tag="qt")
                nc.vector.memset(qt[:, L:QTW], 0.0)
                nc.sync.dma_start_transpose(qt[:, 0:L], q_d[hh])
                kt = iop.tile([64, S], bf16, tag="kt")
                nc.sync.dma_start_transpose(kt[:], k_d[hh])
                # V chunks with a ones column: [128, CH, 65]
                va = iop.tile([128, CH, 65], bf16, tag="va")
                nc.sync.dma_start(
                    va[:, :, 0:64],
                    v_d[hh].rearrange("(c p) e -> p c e", p=128),
                )
                nc.vector.memset(va[:, :, 64:65], 1.0)

                oT = otp.tile([65, S], f32, tag="oT")
                for kk in range(4):
                    nc.tensor.matmul(
                        oT[:, 512 * kk : 512 * (kk + 1)],
                        zc[:],
                        zr[:],
                        start=True,
                        stop=False,
                        skip_group_check=True,
                    )

                for c in range(CH):
                    j0 = 128 * c
                    ktc = kt[:, j0 : j0 + 128]
                    psA = psp.tile([128, WIN_A], f32, tag="psA")
                    psB = psp.tile([128, WIN_B], f32, tag="psB")
                    nc.tensor.matmul(
                        psA[:, 0:WB], ktc, qt[:, j0 : j0 + WB],
                        start=True, stop=True,
                    )
                    nc.tensor.matmul(
                        psA[:, WB:WIN_A], ktc,
                        qt[:, j0 + FARS[0] : j0 + FARS[0] + 128],
                        start=True, stop=True,
                    )
                    nc.tensor.matmul(
                        psB[:, 0:128], ktc,
                        qt[:, j0 + FARS[1] : j0 + FARS[1] + 128],
                        start=True, stop=True,
                    )
                    nc.tensor.matmul(
                        psB[:, 128:256], ktc,
                        qt[:, j0 + FARS[2] : j0 + FARS[2] + 128],
                        start=True, stop=True,
                    )
                    pA = scp.tile([128, WIN_A], bf16, tag="pA")
                    pB = scp.tile([128, WIN_B], bf16, tag="pB")
                    nc.scalar.activation(pA[:], psA[:], AF.Exp, scale=SCALE)
                    nc.scalar.activation(pB[:], psB[:], AF.Exp, scale=SCALE)
                    mo = _MASK_IDX[c] * WTOT
                    nc.vector.tensor_mul(pA[:], pA[:], masks[:, mo : mo + WIN_A])
                    nc.vector.tensor_mul(
                        pB[:], pB[:], masks[:, mo + WIN_A : mo + WTOT]
                    )
                    vac = va[:, c, :]
                    for tile_id, soff, dst, w, stop in _pv_pieces(c):
                        src = pA if tile_id == "A" else pB
                        nc.tensor.matmul(
                            oT[:, dst : dst + w],
                            vac,
                            src[:, soff : soff + w],
                            start=False,
                            stop=stop,
                            skip_group_check=True,
                        )

                # epilogue
                ots = iop.tile([65, S], f32, tag="ots")
                for kk in range(4):
                    nc.scalar.copy(
                        ots[:, 512 * kk : 512 * (kk + 1)],
                        oT[:, 512 * kk : 512 * (kk + 1)],
                    )
                for t in range(CH):
                    tp = psp.tile([128, 65], f32, tag="psA")
                    nc.tensor.transpose(tp[:], ots[:, 128 * t : 128 * t + 128], ident[:])
                    rz = scp.tile([128, 1], f32, tag="rz")
                    nc.vector.reciprocal(rz[:], tp[:, 64:65])
                    of = scp.tile([128, 64], f32, tag="of")
                    nc.scalar.mul(of[:], tp[:, 0:64], rz[:])
                    nc.sync.dma_start(o_d[hh, 128 * t : 128 * t + 128, :], of[:])

    nc.finalize()
    return nc


def _get_nc():
    if "nc" not in _CACHE:
        _CACHE["nc"] = _build_nc()
    return _CACHE["nc"]


# ---------------------------------------------------------------- entrypoint
def kernel(queries, keys, values, attention_mask=None, trace=False):
    from concourse.bass_utils import run_bass_kernel_spmd

    q = np.asarray(queries, dtype=np.float32)
    k = np.asarray(keys, dtype=np.float32)
    v = np.asarray(values, dtype=np.float32)

    # [B, L, H, E] -> [B*H, L, E]
    qp = np.ascontiguousarray(q.transpose(0, 2, 1, 3)).reshape(B * H, L, E)
    kp = np.ascontiguousarray(k.transpose(0, 2, 1, 3)).reshape(B * H, S, E)
    vp = np.ascontiguousarray(v.transpose(0, 2, 1, 3)).reshape(B * H, S, D)
    qb = qp.astype(ml_dtypes.bfloat16)
    kb = kp.astype(ml_dtypes.bfloat16)
    vb = vp.astype(ml_dtypes.bfloat16)
    eye = np.eye(65, dtype=np.float32)

    in_maps = []
    for m in range(NC_CORES):
        s0 = PAIRS_PER_CORE * m
        in_maps.append(
            {
                "q": np.ascontiguousarray(qb[s0 : s0 + PAIRS_PER_CORE]),
                "k": np.ascontiguousarray(kb[s0 : s0 + PAIRS_PER_CORE]),
                "v": np.ascontiguousarray(vb[s0 : s0 + PAIRS_PER_CORE]),
                "masks": _MASKS_NP,
                "ident": eye,
            }
        )

    nc = _get_nc()
    res = run_bass_kernel_spmd(
        nc, in_maps, core_ids=list(range(NC_CORES)), trace=trace
    )
    outs = np.stack([r["out"] for r in res.results])  # [8, 2, L, D]
    o = outs.reshape(B, H, L, D).transpose(0, 2, 1, 3)
    if trace:
        kernel.last_exec_time_ns = res.exec_time_ns
        kernel.last_results = res
    return np.ascontiguousarray(o.astype(np.float32))



# revision 5
# speedup vs baseline: 8.8783x; 8.8783x over previous
"""LogSparseAttention Trainium2 kernel.

B,L,H,E = 2,2048,8,64 ; S,D = 2048,64 ; fp32 in/out.
Shard B*H = 16 (b,h) pairs across 8 cores, 2 pairs/core.

Per (b,h), keys chunked 16 x 128 (j on partitions). Per chunk c the sparse
score windows (i = query index) are packed into one PSUM tile psS[128, W]:
  band  : i in [j0, j0+394)            deltas {0..12,14,18,26,42,74,138, 266}
  522A  : i in [j0+512, j0+640)        block-aligned window holding the d=522
                                       diagonal for keys p in [0,118)
  522B  : i in [j0+640, j0+650)        d=522 tail, keys p in [118,128)
  1034A : i in [j0+1024, j0+1152)      d=1034 diagonal, keys p in [0,118)
  1034B : i in [j0+1152, j0+1162)      d=1034 tail
Windows are clipped at L per chunk (block granularity - no partial windows).

One exp per chunk on ScalarE (no max-subtraction: |score*scale| small),
one 0/1 bf16 mask multiply on VectorE (every chunk's mask is a prefix of
one of two static class tiles).  PV: the masked probs are the STATIONARY
matmul operand (i on output partitions), moving operand is V augmented
with a ones column, accumulating straight into O[i, 0:65] PSUM blocks
(col 64 = softmax Z).  Aligned far windows mean every PV matmul writes
output partitions starting at 0 (PE quadrant rule).  Epilogue per 7-block
group: reciprocal of Z + broadcast multiply on VectorE, bf16 DMA out.
"""

import math

import ml_dtypes
import numpy as np

B, L, H, E = 2, 2048, 8, 64
S, D = 2048, 64
NC_CORES = 8
PAIRS_PER_CORE = 2
CH = L // 128  # 16 chunks
SCALE = 1.0 / math.sqrt(E)

WB = 394          # band window width (deltas up to 266+127 < 394)
WMAX = WB + 128 + 10 + 128 + 10  # 670
GRP = 7           # O blocks per PSUM group tile (7*65*4 = 1820B < 1 bank)
NGRP = (CH + GRP - 1) // GRP  # 3


def _wb(c):
    return min(WB, L - 128 * c)


def _sections(c):
    """Score sections for chunk c: (ps_col, q_col, width). Packed layout."""
    j0 = 128 * c
    out = []
    col = 0

    def add(q0, w, valid):
        nonlocal col
        if valid and w > 0:
            out.append((col, q0, w))
            col += w

    add(j0, _wb(c), True)
    add(j0 + 512, 128, j0 + 640 <= L)
    add(j0 + 640, 10, j0 + 650 <= L)
    add(j0 + 1024, 128, j0 + 1152 <= L)
    add(j0 + 1152, 10, j0 + 1162 <= L)
    return out


def _w_total(c):
    s = _sections(c)
    return s[-1][0] + s[-1][2]


def _pv_pieces(c):
    """PV pieces for chunk c: (ps_col, width, block, stop).

    block = absolute O block index (i in [128*blk, 128*blk + width)).
    Every piece writes output partitions [0, width) - PE-aligned.
    """
    wb = _wb(c)
    pieces = []
    for k, f0 in enumerate((0, 128, 256, 384)):
        w = min(128, wb - f0)
        if w > 0:
            pieces.append((f0, w, c + k, k == 0))
    col = wb
    j0 = 128 * c
    for q0, w, blk in ((j0 + 512, 128, c + 4), (j0 + 640, 10, c + 5),
                      (j0 + 1024, 128, c + 8), (j0 + 1152, 10, c + 9)):
        if q0 + w <= L:
            pieces.append((col, w, blk, False))
            col += w
    return pieces


# ---------------------------------------------------------------- host masks
def _full_mask() -> np.ndarray:
    """Replica of the reference log-sparse mask [L, S] (0/1 float32)."""
    log_l = math.ceil(math.log2(L))
    m = np.zeros((L, S), dtype=np.float32)
    for index in range(L):
        row = np.zeros(S, dtype=np.float32)
        if (S // L) * 2 * log_l > index:
            row[: index + 1] = 1.0
        else:
            idx = index
            while idx >= 0:
                if idx - log_l + 1 < 0:
                    row[:idx] = 1.0
                    break
                row[idx - log_l + 1 : idx + 1] = 1.0
                for i in range(log_l):
                    new_index = idx - log_l + 1 - 2**i
                    if idx - new_index <= L and new_index >= 0:
                        row[new_index] = 1.0
                idx -= L
        m[index] = row
    return m


def _window_masks():
    """Per-chunk [128, W(c)] masks; dedupe into prefix classes.

    Returns (class_tiles [ncls][128, WMAX] bf16, cls_of_chunk list[int]).
    Asserts the windows tile the reference mask exactly.
    """
    mf = _full_mask()
    scatter = np.zeros_like(mf)
    per_c = []
    for c in range(CH):
        j0 = 128 * c
        w = _w_total(c)
        m = np.zeros((128, w), dtype=np.float32)
        for col0, q0, width in _sections(c):
            for q in range(width):
                i = q0 + q
                for p in range(128):
                    j = j0 + p
                    if mf[i, j] == 1.0:
                        m[p, col0 + q] = 1.0
                        scatter[i, j] += 1.0
        per_c.append(m)
    if not np.array_equal(scatter, mf):
        bad = np.argwhere(scatter != mf)
        raise AssertionError(f"windows do not tile reference mask: {bad[:5]}")
    # prefix-dedupe: chunk c uses class k if per_c[c] == class_k[:, :W(c)]
    classes: list[np.ndarray] = []
    cls_of = []
    for m in per_c:
        w = m.shape[1]
        for k, cm in enumerate(classes):
            if cm.shape[1] >= w and np.array_equal(cm[:, :w], m):
                cls_of.append(k)
                break
        else:
            # try extending an existing class (m longer than stored prefix)
            for k, cm in enumerate(classes):
                if cm.shape[1] < w and np.array_equal(cm, m[:, : cm.shape[1]]):
                    classes[k] = m
                    cls_of.append(k)
                    break
            else:
                classes.append(m)
                cls_of.append(len(classes) - 1)
    tiles = np.zeros((128, len(classes), WMAX), dtype=ml_dtypes.bfloat16)
    for k, cm in enumerate(classes):
        tiles[:, k, : cm.shape[1]] = cm.astype(ml_dtypes.bfloat16)
    return tiles, cls_of


_MASK_TILES, _MASK_CLS = _window_masks()
_NCLS = _MASK_TILES.shape[1]


# ---------------------------------------------------------------- bass build
_CACHE = {}


def _build_nc():
    import concourse.bacc as bacc
    import concourse.bass as bass
    import concourse.mybir as mybir
    import concourse.tile as tile

    f32 = mybir.dt.float32
    bf16 = mybir.dt.bfloat16
    AF = mybir.ActivationFunctionType

    nc = bacc.Bacc()
    q_d = nc.dram_tensor("qT", [PAIRS_PER_CORE, E, L], bf16, kind="ExternalInput")
    k_d = nc.dram_tensor("kT", [PAIRS_PER_CORE, E, S], bf16, kind="ExternalInput")
    v_d = nc.dram_tensor("va", [PAIRS_PER_CORE, 128, CH, 65], bf16, kind="ExternalInput")
    m_d = nc.dram_tensor("masks", [128, _NCLS, WMAX], bf16, kind="ExternalInput")
    o_d = nc.dram_tensor("out", [PAIRS_PER_CORE, L, D], bf16, kind="ExternalOutput")

    with tile.TileContext(nc) as tc:
        with (
            tc.tile_pool(name="const", bufs=1) as constp,
            tc.tile_pool(name="io", bufs=2) as iop,
            tc.tile_pool(name="sc", bufs=3) as scp,
            tc.tile_pool(name="ep", bufs=2) as epp,
            tc.tile_pool(name="ps", bufs=2, space=bass.MemorySpace.PSUM) as psp,
            tc.tile_pool(name="og", bufs=1, space=bass.MemorySpace.PSUM) as ogp,
        ):
            masks = constp.tile([128, _NCLS, WMAX], bf16)
            nc.sync.dma_start(masks[:], m_d[:])
            zc = constp.tile([1, 128], bf16)
            nc.vector.memset(zc[:], 0.0)
            zr = constp.tile([1, GRP * 65], bf16)
            nc.vector.memset(zr[:], 0.0)

            for hh in range(PAIRS_PER_CORE):
                qt = iop.tile([E, L], bf16, tag="qt")
                nc.sync.dma_start(qt[:], q_d[hh])
                kt = iop.tile([E, S], bf16, tag="kt")
                nc.sync.dma_start(kt[:], k_d[hh])
                va = iop.tile([128, CH, 65], bf16, tag="va")
                nc.sync.dma_start(va[:], v_d[hh])

                og = [None] * NGRP  # group PSUM tiles, lazily inited

                def ensure_group(g):
                    if og[g] is None:
                        t = ogp.tile([128, GRP, 65], f32, tag=f"og{g}")
                        nblk = min(GRP, CH - GRP * g)
                        nc.tensor.matmul(
                            t[:, 0:nblk, :], zc[:], zr[:, 0 : nblk * 65],
                            start=True, stop=False, skip_group_check=True,
                        )
                        og[g] = t
                    return og[g]

                def emit_pv(c):
                    vac = va[:, c, :]
                    pv_src = pS_of[c]
                    for col0, w, blk, stop in _pv_pieces(c):
                        g, bi = blk // GRP, blk % GRP
                        t = ensure_group(g)
                        nc.tensor.matmul(
                            t[0:w, bi, :],
                            pv_src[:, col0 : col0 + w],
                            vac,
                            start=False, stop=stop, skip_group_check=True,
                        )

                def emit_epilogue(g):
                    t = og[g]
                    nblk = min(GRP, CH - GRP * g)
                    rz = epp.tile([128, nblk, 1], f32, tag=f"rz{g}")
                    nc.vector.reciprocal(rz[:], t[:, 0:nblk, 64:65])
                    of = epp.tile([128, nblk, D], bf16, tag=f"of{g}")
                    nc.vector.tensor_mul(
                        of[:], t[:, 0:nblk, 0:64],
                        rz[:].broadcast_to([128, nblk, D]),
                    )
                    r0 = 128 * GRP * g
                    dst = o_d[hh, r0 : r0 + 128 * nblk, :].rearrange(
                        "(b p) d -> p b d", p=128
                    )
                    nc.sync.dma_start(dst, of[:])

                pS_of = {}
                for c in range(CH):
                    j0 = 128 * c
                    w_tot = _w_total(c)
                    ktc = kt[:, j0 : j0 + 128]
                    psS = psp.tile([128, WMAX], f32, tag="psS")
                    # score matmuls (split at the PSUM 512-col bank boundary)
                    for col0, q0, width in _sections(c):
                        a = col0
                        while a < col0 + width:
                            b = min(col0 + width, 512 if a < 512 else col0 + width)
                            qa = q0 + (a - col0)
                            nc.tensor.matmul(
                                psS[:, a:b], ktc, qt[:, qa : qa + (b - a)],
                                start=True, stop=True,
                            )
                            a = b
                    pS = scp.tile([128, WMAX], bf16, tag="pS")
                    pS_of[c] = pS
                    nc.scalar.activation(
                        pS[:, 0:w_tot], psS[:, 0:w_tot], AF.Exp, scale=SCALE
                    )
                    mo = _MASK_CLS[c]
                    nc.vector.tensor_mul(
                        pS[:, 0:w_tot], pS[:, 0:w_tot], masks[:, mo, 0:w_tot]
                    )
                    if c > 0:
                        emit_pv(c - 1)
                        pS_of.pop(c - 2, None)
                        # block b is complete after chunk b's band head piece;
                        # group g retires with block 7g+6 (chunk 7g+6's PV)
                        done = c - 1
                        if done % GRP == GRP - 1:
                            emit_epilogue(done // GRP)
                emit_pv(CH - 1)
                emit_epilogue(NGRP - 1)  # blocks 14,15 after chunk 15's PV

    nc.finalize()
    return nc


def _get_nc():
    if "nc" not in _CACHE:
        _CACHE["nc"] = _build_nc()
    return _CACHE["nc"]


# ---------------------------------------------------------------- entrypoint
def kernel(queries, keys, values, attention_mask=None, trace=False):
    from concourse.bass_utils import run_bass_kernel_spmd

    q = np.asarray(queries, dtype=np.float32)
    k = np.asarray(keys, dtype=np.float32)
    v = np.asarray(values, dtype=np.float32)

    # [B, L, H, E] -> [B*H, L, E]
    qp = np.ascontiguousarray(q.transpose(0, 2, 1, 3)).reshape(B * H, L, E)
    kp = np.ascontiguousarray(k.transpose(0, 2, 1, 3)).reshape(B * H, S, E)
    vp = np.ascontiguousarray(v.transpose(0, 2, 1, 3)).reshape(B * H, S, D)
    # Q^T / K^T as [B*H, E, L] contiguous bf16
    qT = np.ascontiguousarray(qp.transpose(0, 2, 1)).astype(ml_dtypes.bfloat16)
    kT = np.ascontiguousarray(kp.transpose(0, 2, 1)).astype(ml_dtypes.bfloat16)
    # V chunks with ones column: [B*H, 128, CH, 65]
    va = np.ones((B * H, 128, CH, 65), dtype=ml_dtypes.bfloat16)
    va[:, :, :, 0:64] = vp.reshape(B * H, CH, 128, D).transpose(0, 2, 1, 3)

    in_maps = []
    for m in range(NC_CORES):
        s0 = PAIRS_PER_CORE * m
        sl = slice(s0, s0 + PAIRS_PER_CORE)
        in_maps.append(
            {
                "qT": np.ascontiguousarray(qT[sl]),
                "kT": np.ascontiguousarray(kT[sl]),
                "va": np.ascontiguousarray(va[sl]),
                "masks": _MASK_TILES,
            }
        )

    nc = _get_nc()
    res = run_bass_kernel_spmd(
        nc, in_maps, core_ids=list(range(NC_CORES)), trace=trace
    )
    outs = np.stack([np.asarray(r["out"]) for r in res.results])  # [8, 2, L, D]
    o = outs.reshape(B, H, L, D).transpose(0, 2, 1, 3)
    if trace:
        kernel.last_exec_time_ns = res.exec_time_ns
        kernel.last_results = res
    return np.ascontiguousarray(o.astype(np.float32))


# revision 18
# speedup vs baseline: 8.9542x; 1.0085x over previous
"""LogSparseAttention Trainium2 kernel.

B,L,H,E = 2,2048,8,64 ; S,D = 2048,64 ; fp32 in/out.
Shard B*H = 16 (b,h) pairs across 8 cores, 2 pairs/core.

Per (b,h), keys chunked 16 x 128 (j on partitions). Per chunk c the sparse
score windows (i = query index) are packed into one PSUM tile psS[128, W]:
  band  : i in [j0, j0+394)            deltas {0..12,14,18,26,42,74,138, 266}
  522A  : i in [j0+512, j0+640)        block-aligned window holding the d=522
                                       diagonal for keys p in [0,118)
  522B  : i in [j0+640, j0+650)        d=522 tail, keys p in [118,128)
  1034A : i in [j0+1024, j0+1152)      d=1034 diagonal, keys p in [0,118)
  1034B : i in [j0+1152, j0+1162)      d=1034 tail
Windows are clipped at L per chunk (block granularity - no partial windows).

One exp per chunk on ScalarE (no max-subtraction: |score*scale| small),
one 0/1 bf16 mask multiply on VectorE (every chunk's mask is a prefix of
one of two static class tiles).  PV: the masked probs are the STATIONARY
matmul operand (i on output partitions), moving operand is V augmented
with a ones column, accumulating straight into O[i, 0:65] PSUM blocks
(col 64 = softmax Z).  Aligned far windows mean every PV matmul writes
output partitions starting at 0 (PE quadrant rule).  Epilogue per 7-block
group: reciprocal of Z + broadcast multiply on VectorE, bf16 DMA out.
"""

import math

import ml_dtypes
import numpy as np

B, L, H, E = 2, 2048, 8, 64
S, D = 2048, 64
NC_CORES = 8
PAIRS_PER_CORE = 2
CH = L // 128  # 16 chunks
SCALE = 1.0 / math.sqrt(E)

WB = 394          # band window width (deltas up to 266+127 < 394)
WMAX = WB + 128 + 10 + 128 + 10  # 670
GROUPS = [(0, 6), (6, 6), (12, 3), (15, 1)]  # (start block, nblk)
_G_OF = [g for g, (b0, n) in enumerate(GROUPS) for _ in range(n)]


def _grp_of(b):
    return _G_OF[b]


def _wb(c):
    return min(WB, L - 128 * c)


def _sections(c):
    """Score sections for chunk c: (ps_col, q_col, width). Packed layout."""
    j0 = 128 * c
    out = []
    col = 0

    def add(q0, w, valid):
        nonlocal col
        if valid and w > 0:
            out.append((col, q0, w))
            col += w

    add(j0, _wb(c), True)
    add(j0 + 512, 128, j0 + 640 <= L)
    add(j0 + 640, 10, j0 + 650 <= L)
    add(j0 + 1024, 128, j0 + 1152 <= L)
    add(j0 + 1152, 10, j0 + 1162 <= L)
    return out


def _w_total(c):
    s = _sections(c)
    return s[-1][0] + s[-1][2]


def _pv_pieces(c):
    """PV pieces for chunk c: (ps_col, width, block, stop).

    block = absolute O block index (i in [128*blk, 128*blk + width)).
    Every piece writes output partitions [0, width) - PE-aligned.
    """
    wb = _wb(c)
    pieces = []
    for k, f0 in enumerate((0, 128, 256, 384)):
        w = min(128, wb - f0)
        if w > 0:
            pieces.append((f0, w, c + k, k == 0))
    col = wb
    j0 = 128 * c
    for q0, w, blk in ((j0 + 512, 128, c + 4), (j0 + 640, 10, c + 5),
                      (j0 + 1024, 128, c + 8), (j0 + 1152, 10, c + 9)):
        if q0 + w <= L:
            pieces.append((col, w, blk, False))
            col += w
    return pieces


# ---------------------------------------------------------------- host masks
def _full_mask() -> np.ndarray:
    """Replica of the reference log-sparse mask [L, S] (0/1 float32)."""
    log_l = math.ceil(math.log2(L))
    m = np.zeros((L, S), dtype=np.float32)
    for index in range(L):
        row = np.zeros(S, dtype=np.float32)
        if (S // L) * 2 * log_l > index:
            row[: index + 1] = 1.0
        else:
            idx = index
            while idx >= 0:
                if idx - log_l + 1 < 0:
                    row[:idx] = 1.0
                    break
                row[idx - log_l + 1 : idx + 1] = 1.0
                for i in range(log_l):
                    new_index = idx - log_l + 1 - 2**i
                    if idx - new_index <= L and new_index >= 0:
                        row[new_index] = 1.0
                idx -= L
        m[index] = row
    return m


def _window_masks():
    """Per-chunk [128, W(c)] masks; dedupe into prefix classes.

    Returns (class_tiles [ncls][128, WMAX] bf16, cls_of_chunk list[int]).
    Asserts the windows tile the reference mask exactly.
    """
    mf = _full_mask()
    scatter = np.zeros_like(mf)
    per_c = []
    for c in range(CH):
        j0 = 128 * c
        w = _w_total(c)
        m = np.zeros((128, w), dtype=np.float32)
        for col0, q0, width in _sections(c):
            for q in range(width):
                i = q0 + q
                for p in range(128):
                    j = j0 + p
                    if mf[i, j] == 1.0:
                        m[p, col0 + q] = 1.0
                        scatter[i, j] += 1.0
        per_c.append(m)
    if not np.array_equal(scatter, mf):
        bad = np.argwhere(scatter != mf)
        raise AssertionError(f"windows do not tile reference mask: {bad[:5]}")
    # prefix-dedupe: chunk c uses class k if per_c[c] == class_k[:, :W(c)]
    classes: list[np.ndarray] = []
    cls_of = []
    for m in per_c:
        w = m.shape[1]
        for k, cm in enumerate(classes):
            if cm.shape[1] >= w and np.array_equal(cm[:, :w], m):
                cls_of.append(k)
                break
        else:
            # try extending an existing class (m longer than stored prefix)
            for k, cm in enumerate(classes):
                if cm.shape[1] < w and np.array_equal(cm, m[:, : cm.shape[1]]):
                    classes[k] = m
                    cls_of.append(k)
                    break
            else:
                classes.append(m)
                cls_of.append(len(classes) - 1)
    tiles = np.zeros((128, len(classes), WMAX), dtype=ml_dtypes.bfloat16)
    for k, cm in enumerate(classes):
        tiles[:, k, : cm.shape[1]] = cm.astype(ml_dtypes.bfloat16)
    return tiles, cls_of


_MASK_TILES, _MASK_CLS = _window_masks()
_NCLS = _MASK_TILES.shape[1]


# ---------------------------------------------------------------- bass build
_CACHE = {}


def _build_nc():
    import concourse.bacc as bacc
    import concourse.bass as bass
    import concourse.mybir as mybir
    import concourse.tile as tile

    f32 = mybir.dt.float32
    bf16 = mybir.dt.bfloat16
    AF = mybir.ActivationFunctionType

    nc = bacc.Bacc()
    q_d = nc.dram_tensor("qT", [PAIRS_PER_CORE, E, L], bf16, kind="ExternalInput")
    k_d = nc.dram_tensor("kT", [PAIRS_PER_CORE, E, S], bf16, kind="ExternalInput")
    v_d = nc.dram_tensor("va", [PAIRS_PER_CORE, 128, CH, 65], bf16, kind="ExternalInput")
    m_d = nc.dram_tensor("masks", [128, _NCLS, WMAX], bf16, kind="ExternalInput")
    o_d = nc.dram_tensor("out", [PAIRS_PER_CORE, L, D], bf16, kind="ExternalOutput")

    with tile.TileContext(nc) as tc:
        with (
            tc.tile_pool(name="const", bufs=1) as constp,
            tc.tile_pool(name="io", bufs=2) as iop,
            tc.tile_pool(name="sc", bufs=3) as scp,
            tc.tile_pool(name="ep", bufs=2) as epp,
            tc.tile_pool(name="ps", bufs=2, space=bass.MemorySpace.PSUM) as psp,
            tc.tile_pool(name="og", bufs=1, space=bass.MemorySpace.PSUM) as ogp,
        ):
            masks = constp.tile([128, _NCLS, WMAX], bf16)
            zc = constp.tile([1, 128], bf16)
            nc.vector.memset(zc[:], 0.0)
            zr = constp.tile([1, 6 * 65], bf16)
            nc.vector.memset(zr[:], 0.0)

            # ---- flattened 2-pair x 16-chunk software pipeline ----
            # State per pair: input tiles + O group tiles.
            ctx = {}

            def load_pair(hh):
                kt = iop.tile([E, S], bf16, tag="kt")
                qt = iop.tile([E, L], bf16, tag="qt")
                if hh == 0:
                    # chunk 0 only needs kt[:, 0:128] and qt[:, 0:1162]
                    nc.sync.dma_start(kt[:, 0:256], k_d[hh, :, 0:256])
                    nc.sync.dma_start(qt[:, 0:1280], q_d[hh, :, 0:1280])
                    nc.sync.dma_start(qt[:, 1280:L], q_d[hh, :, 1280:L])
                    nc.sync.dma_start(kt[:, 256:S], k_d[hh, :, 256:S])
                    nc.sync.dma_start(masks[:], m_d[:])
                else:
                    nc.sync.dma_start(qt[:], q_d[hh])
                    nc.sync.dma_start(kt[:], k_d[hh])
                va = iop.tile([128, CH, 65], bf16, tag="va")
                nc.sync.dma_start(va[:], v_d[hh])
                ctx[hh] = {"qt": qt, "kt": kt, "va": va, "og": [None] * len(GROUPS)}

            def ensure_group(hh, g):
                og = ctx[hh]["og"]
                if og[g] is None:
                    nblk = GROUPS[g][1]
                    t = ogp.tile([128, nblk, 65], f32, tag=f"og{g}")
                    nc.tensor.matmul(
                        t[:, :, :], zc[:], zr[:, 0 : nblk * 65],
                        start=True, stop=False, skip_group_check=True,
                    )
                    og[g] = t
                return og[g]

            def emit_pv(vc):
                hh, c = divmod(vc, CH)
                vac = ctx[hh]["va"][:, c, :]
                pv_src = pS_of[vc]
                for col0, w, blk, stop in _pv_pieces(c):
                    g = _grp_of(blk)
                    bi = blk - GROUPS[g][0]
                    t = ensure_group(hh, g)
                    nc.tensor.matmul(
                        t[0:w, bi, :],
                        pv_src[:, col0 : col0 + w],
                        vac,
                        start=False, stop=stop, skip_group_check=True,
                    )

            def emit_epilogue(hh, g):
                t = ctx[hh]["og"][g]
                b0, nblk = GROUPS[g]
                rz = epp.tile([128, nblk, 1], f32, tag=f"rz{g}")
                nc.vector.reciprocal(rz[:], t[:, :, 64:65])
                of = epp.tile([128, nblk, D], bf16, tag=f"of{g}")
                nc.vector.tensor_mul(
                    of[:], t[:, :, 0:64],
                    rz[:].broadcast_to([128, nblk, D]),
                )
                dst = o_d[hh, 128 * b0 : 128 * (b0 + nblk), :].rearrange(
                    "(b p) d -> p b d", p=128
                )
                nc.sync.dma_start(dst, of[:])

            load_pair(0)
            pS_of = {}
            NV = PAIRS_PER_CORE * CH
            for vc in range(NV):
                hh, c = divmod(vc, CH)
                if vc == CH - 2:
                    load_pair(1)  # prefetch pair 1 inputs
                j0 = 128 * c
                w_tot = _w_total(c)
                qt, kt = ctx[hh]["qt"], ctx[hh]["kt"]
                ktc = kt[:, j0 : j0 + 128]
                psS = psp.tile([128, WMAX], f32, tag="psS")
                # score matmuls (split at the PSUM 512-col bank boundary)
                for col0, q0, width in _sections(c):
                    a = col0
                    while a < col0 + width:
                        b = min(col0 + width, 512 if a < 512 else col0 + width)
                        qa = q0 + (a - col0)
                        nc.tensor.matmul(
                            psS[:, a:b], ktc, qt[:, qa : qa + (b - a)],
                            start=True, stop=True,
                        )
                        a = b
                pS = scp.tile([128, WMAX], bf16, tag="pS")
                pS_of[vc] = pS
                nc.scalar.activation(
                    pS[:, 0:w_tot], psS[:, 0:w_tot], AF.Exp, scale=SCALE
                )
                mo = _MASK_CLS[c]
                nc.vector.tensor_mul(
                    pS[:, 0:w_tot], pS[:, 0:w_tot], masks[:, mo, 0:w_tot]
                )
                if vc > 0:
                    emit_pv(vc - 1)
                    pS_of.pop(vc - 2, None)
                    # block b of a pair retires after its chunk-b band head
                    hp, done = divmod(vc - 1, CH)
                    for g, (b0, nblk) in enumerate(GROUPS):
                        if b0 + nblk - 1 == done:
                            emit_epilogue(hp, g)
            emit_pv(NV - 1)
            emit_epilogue(PAIRS_PER_CORE - 1, len(GROUPS) - 1)

    nc.finalize()
    return nc


def _get_nc():
    if "nc" not in _CACHE:
        _CACHE["nc"] = _build_nc()
    return _CACHE["nc"]


# ---------------------------------------------------------------- entrypoint
def kernel(queries, keys, values, attention_mask=None, trace=False):
    from concourse.bass_utils import run_bass_kernel_spmd

    q = np.asarray(queries, dtype=np.float32)
    k = np.asarray(keys, dtype=np.float32)
    v = np.asarray(values, dtype=np.float32)

    # [B, L, H, E] -> [B*H, L, E]
    qp = np.ascontiguousarray(q.transpose(0, 2, 1, 3)).reshape(B * H, L, E)
    kp = np.ascontiguousarray(k.transpose(0, 2, 1, 3)).reshape(B * H, S, E)
    vp = np.ascontiguousarray(v.transpose(0, 2, 1, 3)).reshape(B * H, S, D)
    # Q^T / K^T as [B*H, E, L] contiguous bf16
    qT = np.ascontiguousarray(qp.transpose(0, 2, 1)).astype(ml_dtypes.bfloat16)
    kT = np.ascontiguousarray(kp.transpose(0, 2, 1)).astype(ml_dtypes.bfloat16)
    # V chunks with ones column: [B*H, 128, CH, 65]
    va = np.ones((B * H, 128, CH, 65), dtype=ml_dtypes.bfloat16)
    va[:, :, :, 0:64] = vp.reshape(B * H, CH, 128, D).transpose(0, 2, 1, 3)

    in_maps = []
    for m in range(NC_CORES):
        s0 = PAIRS_PER_CORE * m
        sl = slice(s0, s0 + PAIRS_PER_CORE)
        in_maps.append(
            {
                "qT": np.ascontiguousarray(qT[sl]),
                "kT": np.ascontiguousarray(kT[sl]),
                "va": np.ascontiguousarray(va[sl]),
                "masks": _MASK_TILES,
            }
        )

    nc = _get_nc()
    res = run_bass_kernel_spmd(
        nc, in_maps, core_ids=list(range(NC_CORES)), trace=trace
    )
    outs = np.stack([np.asarray(r["out"]) for r in res.results])  # [8, 2, L, D]
    o = outs.reshape(B, H, L, D).transpose(0, 2, 1, 3)
    if trace:
        kernel.last_exec_time_ns = res.exec_time_ns
        kernel.last_results = res
    return np.ascontiguousarray(o.astype(np.float32))
